# revision 15
# baseline (speedup 1.0000x reference)
"""Trainium2 Bass kernel for BC_Encoder (MLP + segmented mean/max/min pooling).

Strategy (8-core SPMD, identical program on every core):
  - Host packs each core's ~N/8 points into segment-pure 512-point tiles
    (tiles never straddle a segment boundary; short tiles are padded by
    replicating the tile's first point).
  - Device per tile: L1 (K=3) -> LN -> ReLU -> L2 -> LN -> ReLU -> L3,
    with point-major matmuls (lhsT = feature-major activations, fp32r),
    LN stats via bn_stats (per-point = per-partition), mean/rstd folded
    into the ScalarE eviction, PE transpose to feature-major where gamma/
    beta/ReLU are per-partition, L3 feature-major so pooling reduces run
    along the free axis.  Per tile the device emits per-tile sum/max/min
    columns plus the tile's first-point feature vector (for the host-side
    replicate-padding correction).
  - Host un-pads (sum -= n_pad * col0), combines tiles into segments,
    reduces across cores, divides by true counts, adds b3, concatenates.
"""

import numpy as np

N_CORES = 8
DIN = 3
DINA = 4  # DIN + a constant-ones row carrying b1
H = 256
EPS = 1e-5
TILE = 512
PB = 128
NPB = TILE // PB  # point-blocks per tile

_PROGRAM_CACHE = {}


def _build_program(nt):
    import concourse.bass as bass
    import concourse.tile as tile
    from concourse import bacc, mybir
    from concourse.masks import make_identity

    f32 = mybir.dt.float32
    f16 = mybir.dt.float16
    f32r = mybir.dt.float32r

    nc = bacc.Bacc("TRN2", target_bir_lowering=False, debug=False)

    posT = nc.dram_tensor("posT", [DINA, nt * TILE], f32r, kind="ExternalInput")
    w1t = nc.dram_tensor("w1t", [DINA, H], f32r, kind="ExternalInput")
    w2t = nc.dram_tensor("w2t", [H, H], f32r, kind="ExternalInput")
    w3t = nc.dram_tensor("w3t", [H, H], f32r, kind="ExternalInput")
    b2r = nc.dram_tensor("b2r", [1, H], f32r, kind="ExternalInput")
    onesr = nc.dram_tensor("onesr", [1, PB], f32r, kind="ExternalInput")
    gbe = nc.dram_tensor("gbe", [H, 4], f32, kind="ExternalInput")
    stag_d = nc.dram_tensor("stag", [8, PB, nt], f32, kind="ExternalOutput")

    def r(ap):
        return ap if ap.dtype == f32r else ap.bitcast(f32r)

    with tile.TileContext(nc) as tc:
        with (
            tc.tile_pool(name="consts", bufs=1) as consts,
            tc.tile_pool(name="xin", bufs=4) as xin,
            tc.tile_pool(name="tsb", bufs=2) as tsb,
            tc.tile_pool(name="zsb", bufs=3) as zsb,
            tc.tile_pool(name="stats", bufs=4) as stats_p,
            tc.tile_pool(name="psy", bufs=2, space="PSUM") as psy,
            tc.tile_pool(name="pstt", bufs=4, space="PSUM") as pstt,
        ):
            # ---- constants ----
            w1_sb = consts.tile([DINA, H], f32r)
            nc.sync.dma_start(w1_sb[:], w1t[:])
            b2_sb = consts.tile([1, H], f32r)
            nc.sync.dma_start(b2_sb[:], b2r[:])
            ones1 = consts.tile([1, PB], f32r)
            nc.sync.dma_start(ones1[:], onesr[:])
            w2_sb = [consts.tile([PB, H], f32r, tag=f"w2_{k}", name=f"w2_{k}") for k in range(2)]
            for k in range(2):
                nc.sync.dma_start(w2_sb[k][:], w2t[k * PB : (k + 1) * PB, :])
            w3_sb = [
                [consts.tile([PB, PB], f32r, tag=f"w3_{k}{m}", name=f"w3_{k}{m}") for m in range(2)]
                for k in range(2)
            ]
            for k in range(2):
                for m in range(2):
                    nc.sync.dma_start(
                        w3_sb[k][m][:],
                        w3t[k * PB : (k + 1) * PB, m * PB : (m + 1) * PB],
                    )
            gbe_sb = [consts.tile([PB, 4], f32, tag=f"gbe_{fb}", name=f"gbe_{fb}") for fb in range(2)]
            for fb in range(2):
                nc.sync.dma_start(gbe_sb[fb][:], gbe[fb * PB : (fb + 1) * PB, :])
            eps_sb = consts.tile([PB, 1], f32)
            nc.vector.memset(eps_sb[:], EPS)
            ident = consts.tile([PB, PB], f16)
            make_identity(nc, ident[:])
            # staging accumulators (written column-by-column, DMA'd at end)
            stag = [consts.tile([PB, nt], f32, tag=f"stag_{i}", name=f"stag_{i}") for i in range(8)]

            def layer_norm(y_ps, gbe_cols, z_out):
                """y_ps: PSUM [PB, NPB, H] point-major. Writes z_out [PB, 2, TILE]
                feature-major = relu(LN(y) * g + be)."""
                st = stats_p.tile([PB, NPB, 6], f32, tag="bn6")
                nc.vector.bn_stats(st[:], y_ps[:])
                mv = stats_p.tile([PB, NPB, 2], f32, tag="mv")
                for pb in range(NPB):
                    nc.vector.bn_aggr(mv[:, pb, :], st[:, pb, :])
                rstd = stats_p.tile([PB, NPB], f32, tag="rstd")
                nc.scalar.activation(
                    rstd[:], mv[:, :, 1], mybir.ActivationFunctionType.Sqrt,
                    bias=eps_sb[:], scale=1.0,
                )
                nc.vector.reciprocal(rstd[:], rstd[:])
                nmr = stats_p.tile([PB, NPB], f32, tag="nmr")
                nc.vector.tensor_mul(nmr[:], mv[:, :, 0], rstd[:])
                nc.vector.tensor_scalar_mul(nmr[:], nmr[:], -1.0)
                # evict with per-point (partition) normalization, fp16 out;
                # split across ScalarE (scale/bias form) and VectorE (2-op form)
                t_sb = tsb.tile([PB, NPB, H], f16, tag="t")
                for pb in range(NPB):
                    if pb % 2 == 0:
                        nc.scalar.activation(
                            t_sb[:, pb, :], y_ps[:, pb, :],
                            mybir.ActivationFunctionType.Identity,
                            bias=nmr[:, pb : pb + 1], scale=rstd[:, pb : pb + 1],
                        )
                    else:
                        nc.vector.tensor_scalar(
                            t_sb[:, pb, :], y_ps[:, pb, :],
                            mv[:, pb, 0:1], rstd[:, pb : pb + 1],
                            mybir.AluOpType.subtract, mybir.AluOpType.mult,
                        )
                # transpose to feature-major, then gamma/beta/relu eviction
                for fb in range(2):
                    tt = pstt.tile([PB, TILE], f32, tag="tt")
                    for pb in range(NPB):
                        nc.tensor.transpose(
                            tt[:, pb * PB : (pb + 1) * PB],
                            t_sb[:, pb, fb * PB : (fb + 1) * PB],
                            ident[:],
                        )
                    nc.scalar.activation(
                        z_out[:, fb, :], tt[:],
                        mybir.ActivationFunctionType.Relu,
                        bias=gbe_cols[fb][1], scale=gbe_cols[fb][0],
                    )

            for t in range(nt):
                x0 = xin.tile([DINA, TILE], f32r, tag="x0")
                nc.sync.dma_start(x0[:], posT[:, t * TILE : (t + 1) * TILE])

                # ---- L1 (point-major, K=4: xyz + ones row carrying b1) ----
                y1 = psy.tile([PB, NPB, H], f32, tag="y")
                for pb in range(NPB):
                    nc.tensor.matmul(
                        y1[:, pb, :], r(x0[:, pb * PB : (pb + 1) * PB]), r(w1_sb[:]),
                        start=True, stop=True,
                    )
                z1 = zsb.tile([PB, 2, TILE], f32r, tag="z")
                layer_norm(
                    y1,
                    [(gbe_sb[fb][:, 0:1], gbe_sb[fb][:, 1:2]) for fb in range(2)],
                    z1,
                )

                # ---- L2 (point-major, K=256 in two chunks; b2 via K=1 init) ----
                y2 = psy.tile([PB, NPB, H], f32, tag="y")
                for pb in range(NPB):
                    nc.tensor.matmul(
                        y2[:, pb, :], r(ones1[:]), r(b2_sb[:]),
                        start=True, stop=False,
                    )
                    for k in range(2):
                        nc.tensor.matmul(
                            y2[:, pb, :],
                            r(z1[:, k, pb * PB : (pb + 1) * PB]),
                            r(w2_sb[k][:]),
                            start=False, stop=(k == 1),
                        )
                z2 = zsb.tile([PB, 2, TILE], f32r, tag="z")
                layer_norm(
                    y2,
                    [(gbe_sb[fb][:, 2:3], gbe_sb[fb][:, 3:4]) for fb in range(2)],
                    z2,
                )

                # ---- L3 (feature-major: out [h-block, pts]) ----
                y3 = [pstt.tile([PB, TILE], f32, tag="tt", name=f"y3_{m}") for m in range(2)]
                for m in range(2):
                    for k in range(2):
                        nc.tensor.matmul(
                            y3[m][:], r(w3_sb[k][m][:]), r(z2[:, k, :]),
                            start=(k == 0), stop=(k == 1),
                        )

                # ---- per-tile pooling columns ----
                X = mybir.AxisListType.X
                for m in range(2):
                    nc.vector.tensor_reduce(
                        stag[0 + m][:, t : t + 1], y3[m][:], axis=X,
                        op=mybir.AluOpType.add,
                    )
                    nc.vector.tensor_reduce(
                        stag[2 + m][:, t : t + 1], y3[m][:], axis=X,
                        op=mybir.AluOpType.max,
                    )
                    nc.vector.tensor_reduce(
                        stag[4 + m][:, t : t + 1], y3[m][:], axis=X,
                        op=mybir.AluOpType.min,
                    )
                    nc.vector.tensor_copy(stag[6 + m][:, t : t + 1], y3[m][:, 0:1])

            for i in range(8):
                nc.sync.dma_start(stag_d[i], stag[i][:])

    nc.compile()
    return nc


def _host_prep(positions, batch_index, n_cores):
    """Pack points into segment-pure tiles per core.

    Returns per-core (index_array [nt*TILE], tmap [nt], n_real [nt]) and nt."""
    n = positions.shape[0]
    bi = np.asarray(batch_index)
    edges = [c * n // n_cores for c in range(n_cores + 1)]
    cores = []
    for c in range(n_cores):
        lo, hi = edges[c], edges[c + 1]
        # segment-run boundaries inside [lo, hi)
        segs = bi[lo:hi]
        cuts = np.flatnonzero(np.diff(segs)) + 1 + lo
        bounds = np.concatenate([[lo], cuts, [hi]])
        idx_parts = []
        tmap = []
        n_real = []
        for j in range(len(bounds) - 1):
            s, e = int(bounds[j]), int(bounds[j + 1])
            seg = int(bi[s])
            for ts in range(s, e, TILE):
                te = min(ts + TILE, e)
                k = te - ts
                part = np.arange(ts, te, dtype=np.int64)
                if k < TILE:
                    part = np.concatenate(
                        [part, np.full(TILE - k, ts, dtype=np.int64)]
                    )
                idx_parts.append(part)
                tmap.append(seg)
                n_real.append(k)
        cores.append((idx_parts, tmap, n_real))
    nt = max(len(cc[1]) for cc in cores)
    out = []
    for idx_parts, tmap, n_real in cores:
        pad_tiles = nt - len(tmap)
        if pad_tiles:
            idx_parts += [np.zeros(TILE, dtype=np.int64)] * pad_tiles
            tmap += [-1] * pad_tiles
            n_real += [0] * pad_tiles
        out.append(
            (
                np.concatenate(idx_parts),
                np.asarray(tmap, np.int64),
                np.asarray(n_real, np.int64),
            )
        )
    return out, nt


def kernel(
    positions, W1, b1, W2, b2, W3, b3, g1, be1, g2, be2, batch_index, num_segments
):
    from concourse.bass_utils import run_bass_kernel_spmd

    positions = np.asarray(positions, np.float32)
    W1 = np.asarray(W1, np.float32)
    b1 = np.asarray(b1, np.float32)
    W2 = np.asarray(W2, np.float32)
    b2 = np.asarray(b2, np.float32)
    W3 = np.asarray(W3, np.float32)
    b3 = np.asarray(b3, np.float32)
    g1 = np.asarray(g1, np.float32)
    be1 = np.asarray(be1, np.float32)
    g2 = np.asarray(g2, np.float32)
    be2 = np.asarray(be2, np.float32)
    bi = np.asarray(batch_index)
    B = int(num_segments)

    cores, nt = _host_prep(positions, bi, N_CORES)

    if nt not in _PROGRAM_CACHE:
        _PROGRAM_CACHE[nt] = _build_program(nt)
    nc = _PROGRAM_CACHE[nt]

    # b1 rides as the 4th row of w1t against a constant-ones input row;
    # b2 is added on-device via a K=1 PSUM-init matmul; b3 is added on host.
    w1t = np.ascontiguousarray(np.concatenate([W1.T, b1[None, :]], axis=0))  # [4, H]
    w2t = np.ascontiguousarray(W2.T)  # [H, H]
    w3t = np.ascontiguousarray(W3.T)  # [H, H]
    b2r = np.ascontiguousarray(b2[None, :])  # [1, H]
    gbe = np.ascontiguousarray(np.stack([g1, be1, g2, be2], axis=1))  # [H, 4]

    in_maps = []
    for idx, tmap, n_real in cores:
        pos_aug = np.empty((DINA, idx.shape[0]), np.float32)
        pos_aug[:DIN] = positions[idx].T
        pos_aug[DIN] = 1.0
        in_maps.append(
            {
                "posT": pos_aug,
                "w1t": w1t,
                "w2t": w2t,
                "w3t": w3t,
                "b2r": b2r,
                "onesr": np.ones((1, PB), np.float32),
                "gbe": gbe,
            }
        )

    res = run_bass_kernel_spmd(nc, in_maps, list(range(N_CORES)))

    # ---- host-side unshard / segment combine ----
    sums = np.zeros((H, B), np.float64)
    maxs = np.full((H, B), -np.inf, np.float32)
    mins = np.full((H, B), np.inf, np.float32)
    for c, (idx, tmap, n_real) in enumerate(cores):
        stag = res.results[c]["stag"]  # [8, PB, nt]
        s_all = np.concatenate([stag[0], stag[1]], axis=0)  # [H, nt]
        mx_all = np.concatenate([stag[2], stag[3]], axis=0)
        mn_all = np.concatenate([stag[4], stag[5]], axis=0)
        c0_all = np.concatenate([stag[6], stag[7]], axis=0)
        npad = (TILE - n_real).astype(np.float64)
        live = n_real > 0
        s_corr = s_all.astype(np.float64) - npad[None, :] * c0_all.astype(np.float64)
        for t in np.flatnonzero(live):
            seg = tmap[t]
            sums[:, seg] += s_corr[:, t]
            np.maximum(maxs[:, seg], mx_all[:, t], out=maxs[:, seg])
            np.minimum(mins[:, seg], mn_all[:, t], out=mins[:, seg])

    counts = np.bincount(bi.astype(np.int64), minlength=B).astype(np.float64)
    mean_p = (sums / counts[None, :]).T.astype(np.float32) + b3[None, :]
    max_p = maxs.T + b3[None, :]
    min_p = mins.T + b3[None, :]
    return np.concatenate([mean_p, max_p, min_p], axis=1).astype(np.float32)
